# revision 26
# baseline (speedup 1.0000x reference)
"""BERT self-attention (B=4, S=2048, D=1024, H=16) on 8 trn2 NeuronCores.

Sharding: core c -> (batch b = c//2, head-group hg = c%2, 8 heads each).
Each core computes out[b, :, hg*512:(hg+1)*512] independently; host
gathers. Inputs are pre-transposed on host so the contraction dim (d)
lands on SBUF partitions: xt = X.T [D,S], w{q,k,v}t = W.T shard [D,512].

v2 design (ACT-bound pipeline, ~all engines overlapped):
  - Q^T/K^T pair-tiles [128, S] f32r (2 heads per tile, dh on partitions).
  - V_aug [128j, 8h, 65] bf16 per s-tile: V + bias, col 64 = ones (gives
    the softmax denominator for free during the ctx matmul).
  - Attention per (pair p, query-quarter qc): 16 j-tiles; scores for the
    2 heads go to one [128, 2, 512] PSUM tile (2 banks) via concurrent
    row-group matmuls; ONE exp per jt ([128,1024] ACT op, mask as bias);
    ctx accumulated IN PSUM across all 16 jt (C[65,512] per head) - no
    DVE adds in the inner loop.
  - Drain: C -> SBUF copy, PE-transpose 128-blocks, reciprocal of the
    denominator row, scale, DMA out.
  - V and all QK projections run contiguously up front (interleaving
    proj chunks into attention measured worse: aux-slot contention
    inflates proj matmuls 247->403ns and triples exp gaps).
PSUM budget: sp0(2) + sp1(2) + c0(1) + c1(1) + aux(2) = 8 banks.
"""

import numpy as np

import concourse.bass as bass
import concourse.tile as tile
from concourse import bacc, mybir
from concourse.bass_utils import run_bass_kernel_spmd
from concourse.masks import make_identity

B, S, D, H = 4, 2048, 1024, 16
DH = 64
O = 512  # per-core output width (8 heads)
HL = 8  # local heads per core
NP = 4  # head pairs per core
ST = S // 128  # 16 s-tiles
QC = 4  # query quarters (512 queries each)
F32 = mybir.dt.float32
F32R = mybir.dt.float32r
BF16 = mybir.dt.bfloat16
EXP = mybir.ActivationFunctionType.Exp

_NC_CACHE = None


def build_nc():
    nc = bacc.Bacc(
        "TRN2",
        target_bir_lowering=False,
        debug=False,
        enable_asserts=True,
        num_devices=8,
    )
    xt = nc.dram_tensor("xt", [D, S], F32R, kind="ExternalInput").ap()
    wqt = nc.dram_tensor("wqt", [D, O], F32R, kind="ExternalInput").ap()
    wkt = nc.dram_tensor("wkt", [D, O], F32R, kind="ExternalInput").ap()
    wvt = nc.dram_tensor("wvt", [D, O], F32R, kind="ExternalInput").ap()
    bq = nc.dram_tensor("bq", [O], F32, kind="ExternalInput").ap()
    bk = nc.dram_tensor("bk", [O], F32, kind="ExternalInput").ap()
    bv = nc.dram_tensor("bv", [O], F32, kind="ExternalInput").ap()
    mask = nc.dram_tensor("mask", [S], F32, kind="ExternalInput").ap()
    out = nc.dram_tensor("out", [S, O], F32, kind="ExternalOutput").ap()

    with tile.TileContext(nc) as tc:
        _emit(nc, tc, xt, wqt, wkt, wvt, bq, bk, bv, mask, out)
    nc.compile()
    return nc


def _emit(nc, tc, xt, wqt, wkt, wvt, bq, bk, bv, mask, out):
    with (
        tc.tile_pool(name="singles", bufs=1) as singles,
        tc.tile_pool(name="persist", bufs=1) as persist,
        tc.tile_pool(name="wpool", bufs=1) as wpool,
        tc.tile_pool(name="attn", bufs=1) as attn,
        tc.tile_pool(name="psum", bufs=1, space="PSUM") as psum,
    ):
        ident = singles.tile([128, 128], F32)
        make_identity(nc, ident)
        mask_sb = singles.tile([128, ST], F32)
        nc.sync.dma_start(out=mask_sb, in_=mask.rearrange("(t p) -> p t", p=128))
        bq_sb = singles.tile([128, NP], F32)
        nc.sync.dma_start(out=bq_sb, in_=bq.rearrange("(t p) -> p t", p=128))
        bk_sb = singles.tile([128, NP], F32)
        nc.sync.dma_start(out=bk_sb, in_=bk.rearrange("(t p) -> p t", p=128))
        bv_bc = singles.tile([128, HL, DH], F32)
        nc.sync.dma_start(
            out=bv_bc, in_=bass.AP(tensor=bv.tensor, offset=0, ap=[[0, 128], [1, O]])
        )

        # persistent activations
        xts = [persist.tile([128, S], F32R, name=f"xts{dt}", tag=f"xts{dt}") for dt in range(8)]
        qts = [persist.tile([128, S], F32R, name=f"qt{p}", tag=f"qt{p}") for p in range(NP)]
        kts = [persist.tile([128, S], F32R, name=f"kt{p}", tag=f"kt{p}") for p in range(NP)]
        vaug = [
            persist.tile([128, HL, DH + 1], BF16, name=f"vaug{t}", tag=f"vaug{t}")
            for t in range(ST)
        ]

        for dt in range(8):
            nc.sync.dma_start(out=xts[dt], in_=xt[dt * 128 : (dt + 1) * 128, :])

        # ---- V projection (all heads, up front) ----
        wv_t = []
        for dt in range(8):
            w = wpool.tile([128, O], F32R, name=f"wv{dt}", tag="wv", bufs=8)
            nc.sync.dma_start(out=w, in_=wvt[dt * 128 : (dt + 1) * 128, :])
            wv_t.append(w)
        for st in range(ST):
            ps = psum.tile([128, HL, DH], F32, name=f"psv{st}", tag=f"sp{st % 2}", bufs=1)
            for dt in range(8):
                nc.tensor.matmul(
                    ps,
                    xts[dt][:, st * 128 : (st + 1) * 128],
                    wv_t[dt],
                    start=(dt == 0),
                    stop=(dt == 7),
                )
            va = vaug[st]
            nc.vector.memset(va[:, :, DH : DH + 1], 1.0)
            nc.vector.tensor_add(va[:, :, 0:DH], ps, bv_bc)

        # ---- Q/K projection machinery (per-pair, chunked) ----
        wslices = {}  # (which, p) -> list of 8 [128,128] tiles

        def load_w_slices(which, p):
            wdram = {"k": wkt, "q": wqt}[which]
            tiles = []
            for dt in range(8):
                w = wpool.tile(
                    [128, 128], F32R, name=f"w{which}{p}_{dt}", tag=f"w{which}", bufs=8
                )
                nc.sync.dma_start(
                    out=w,
                    in_=wdram[dt * 128 : (dt + 1) * 128, p * 128 : (p + 1) * 128],
                )
                tiles.append(w)
            wslices[which, p] = tiles

        def emit_qk_chunk(which, p, c):
            if (which, p) not in wslices:
                load_w_slices(which, p)
            wts = wslices[which, p]
            dst = {"k": kts, "q": qts}[which][p]
            bias_sb = {"k": bk_sb, "q": bq_sb}[which]
            ps = psum.tile([128, 512], F32, name=f"ps{which}{p}_{c}", tag=f"sp{c % 2}", bufs=1)
            for dt in range(8):
                nc.tensor.matmul(
                    ps,
                    wts[dt],
                    xts[dt][:, c * 512 : (c + 1) * 512],
                    start=(dt == 0),
                    stop=(dt == 7),
                )
            nc.vector.tensor_scalar_add(
                dst[:, c * 512 : (c + 1) * 512], ps, bias_sb[:, p : p + 1]
            )

        # All QK projections up front, contiguously: interleaving them into
        # attention's PE slack measured WORSE (proj matmuls inflate 247->403ns
        # on aux-slot contention and exp gaps triple in chunked blocks).
        for p in range(NP):
            for which in ("k", "q"):
                for c in range(4):
                    emit_qk_chunk(which, p, c)

        # ---- attention ----
        for p in range(NP):
            ktp, qtp = kts[p], qts[p]
            for qc in range(QC):
                base = qc * 512
                C = [
                    psum.tile(
                        [DH + 1, 512], F32, name=f"c{x}_{p}_{qc}", tag=f"c{x}", bufs=1
                    )
                    for x in range(2)
                ]
                for jt in range(ST):
                    sp = psum.tile(
                        [128, 2, 512],
                        F32,
                        name=f"sp{p}_{qc}_{jt}",
                        tag=f"sp{jt % 3}",
                        bufs=1,
                    )
                    for x in range(2):
                        hp = slice(x * 64, x * 64 + 64)
                        nc.tensor.matmul(
                            sp[:, x, :],
                            ktp[hp, jt * 128 : (jt + 1) * 128],
                            qtp[hp, base : base + 512],
                            start=True,
                            stop=True,
                        )
                    u = attn.tile(
                        [128, 2, 512],
                        BF16,
                        name=f"u{p}_{qc}_{jt}",
                        tag=f"u{jt % 3}",
                        bufs=1,
                    )
                    nc.scalar.activation(
                        u, sp, EXP, bias=mask_sb[:, jt : jt + 1], scale=0.125
                    )
                    for x in range(2):
                        nc.tensor.matmul(
                            C[x],
                            vaug[jt][:, 2 * p + x, :],
                            u[:, x, :],
                            start=(jt == 0),
                            stop=(jt == ST - 1),
                        )
                # drain: copy to SBUF, PE-transpose 128-blocks, then
                # normalize by the denominator row and store.
                for x in range(2):
                    hh = 2 * p + x
                    csb = attn.tile(
                        [DH + 1, 512], F32, name=f"csb{p}_{qc}_{x}", tag=f"csb{x}",
                        bufs=2,
                    )
                    nc.vector.tensor_copy(out=csb, in_=C[x])
                    for it in range(4):
                        tp_ = psum.tile(
                            [128, DH + 1], F32, name=f"tp{p}_{qc}_{x}_{it}",
                            tag=f"c{x}", bufs=1,
                        )
                        nc.tensor.transpose(
                            tp_,
                            csb[:, it * 128 : (it + 1) * 128],
                            ident[0 : DH + 1, 0 : DH + 1],
                        )
                        rc = attn.tile(
                            [128, 1], F32, name=f"rc{p}_{qc}_{x}_{it}", tag="rc", bufs=6
                        )
                        nc.vector.reciprocal(rc, tp_[:, DH : DH + 1])
                        ot = attn.tile(
                            [128, DH], F32, name=f"ot{p}_{qc}_{x}_{it}", tag="ot", bufs=6
                        )
                        nc.vector.tensor_scalar_mul(ot, tp_[:, 0:DH], rc)
                        row = base + it * 128
                        nc.sync.dma_start(
                            out=out[row : row + 128, hh * DH : (hh + 1) * DH], in_=ot
                        )


def _make_in_maps(hidden_states, attention_mask, Wq, bq, Wk, bk, Wv, bv):
    in_maps = []
    for c in range(8):
        b, hg = divmod(c, 2)
        sl = slice(hg * O, (hg + 1) * O)
        in_maps.append(
            {
                "xt": np.ascontiguousarray(hidden_states[b].T),
                "wqt": np.ascontiguousarray(Wq[sl, :].T),
                "wkt": np.ascontiguousarray(Wk[sl, :].T),
                "wvt": np.ascontiguousarray(Wv[sl, :].T),
                "bq": np.ascontiguousarray(bq[sl]),
                "bk": np.ascontiguousarray(bk[sl]),
                "bv": np.ascontiguousarray(bv[sl]),
                "mask": np.ascontiguousarray(attention_mask[b, 0, 0, :]),
            }
        )
    return in_maps


def _gather(results):
    out = np.empty((B, S, D), dtype=np.float32)
    for c in range(8):
        b, hg = divmod(c, 2)
        out[b, :, hg * O : (hg + 1) * O] = results[c]["out"]
    return out


def kernel(hidden_states, attention_mask, Wq, bq, Wk, bk, Wv, bv, **run_kwargs):
    global _NC_CACHE
    args = [hidden_states, attention_mask, Wq, bq, Wk, bk, Wv, bv]
    args = [np.asarray(a, dtype=np.float32) for a in args]
    if _NC_CACHE is None:
        _NC_CACHE = build_nc()
    in_maps = _make_in_maps(*args)
    res = run_bass_kernel_spmd(_NC_CACHE, in_maps, core_ids=list(range(8)), **run_kwargs)
    kernel.last_result = res
    return _gather(res.results)
